# revision 15
# baseline (speedup 1.0000x reference)
"""BSA kernel v6: single custom-DVE instruction per K time steps.

Reformulation: keep the tree-computed window sums A0 (minus L) in a t-major
interleaved buffer X[p, (20+t)*8+g]; the 20 leading slots per group are
zero-padded. Step t's mask is m[t] = (c[t] <= A0[t]-L) with
c[t] = sum_{j=1..20} m[t-j]*G[j-1]. One custom DVE op ("BSA_STEP_ANT")
computes, per page of 21 elements read with inner stride -8
([A0L[t], m[t-1], ..., m[t-20]]), a segmented scan-add of products with
weights [-1, G[0..19]] and writes (scan <= 0) with a stride-0 output AP, so
the last write lands m[t] in place of A0L[t]. Pages enumerate (t, g) with
start stride +1, so one instruction runs K steps x 8 groups (scan resets at
every SUB_DIM boundary via a hand-built uop step state). The 8-page distance
between consecutive same-group steps keeps the in-flight write->read gap
(~141 cycles) above the SBUF write-visibility latency; tighter interleaves
(pair/group-major) read stale data. Host does layout only.
"""
import numpy as np

B, T, F = 8192, 2048, 20
NSTEPS = T - F                  # 2028
NCORES = 8
RPC = B // NCORES               # 1024
NG = RPC // 128                 # 8
THRESHOLD = 0.9952
CH = 256                        # chunk size (steps)
NCH = T // CH                   # 8 sig chunks
HALO = 34                       # tree halo in steps
PAD = F                         # zero history slots per group
KSTEP = 256                     # time steps fused per chain instruction

_CACHE = {}

# ---------------------------------------------------------------- custom op
OP_NAME = "BSA_STEP_ANT"


def _register_op():
    """Define + register the segmented-scan BSA step op. Idempotent."""
    from concourse import dve_ops as _dvo
    from concourse.dve_spec import (
        Spec, Src0, Src1, Zero, scan,
        _validate_body, _hoist_stream_invariant_ops, _collect,
        _build_placement, _build_state_machine, _assemble, _State, _Stage,
        Scan, Latch,
    )
    from concourse.dve_uop import AluOp, Trigger, DveOpSpec, N_LANES, N_STAGES

    for op in _dvo.OPS:
        if op.name == OP_NAME:
            return op

    def _ref(in0, in1, c0, c1, c2):
        p = in0.astype(np.float32) * in1.astype(np.float32)
        r = np.cumsum(p, axis=-1)
        return (r <= 0.0).astype(np.float32)

    body = scan(AluOp.ADD, Src0 * Src1) <= Zero
    spec = Spec(body=body, reference=_ref)
    ver = "v3"
    _validate_body(spec, ver)
    spec2 = _hoist_stream_invariant_ops(spec)
    scans = _collect(spec2.body, Scan)
    latches = _collect(spec2.body, Latch)
    assert len(scans) == 1 and not latches
    placement = _build_placement(spec2, scans, N_STAGES[ver], N_LANES[ver])
    stock = _build_state_machine(spec2, scans, latches, placement)
    assert len(stock) == 2
    seed = stock[0]
    sc = scans[0]
    d = placement.node_stage[sc]
    consume = stock[1].consume
    steady = _State(
        placement=placement, consume=consume,
        trigger=(Trigger.SRC_TENSOR_DONE, Trigger.SUB_DIM_DONE, Trigger.NONE),
        next=(0, 2, 0),
    )
    step = _State(
        placement=placement, consume=consume,
        overrides={d: _Stage(AluOp.ADD, Zero, sc.expr)},
        trigger=(Trigger.SRC_TENSOR_DONE, Trigger.SUB_DIM_DONE, Trigger.COUNT),
        next=(0, 2, 1),
        repeat=1,
    )
    uops = [_assemble(s) for s in (seed, steady, step)]
    for u in uops:
        u.validate(ver)
    row = _dvo._CUSTOM_DVE_ROW_BASE + len(_dvo.OPS)
    assert row < 0x20
    _dvo._SUB_OPCODE_FOR_NAME[OP_NAME] = row
    compiled = DveOpSpec(name=OP_NAME, opcode=row, uops=uops, rd1_en=True)
    op = _dvo.DveOp(
        OP_NAME, spec, subdim=True,
        uops_sha={"v3": compiled.sha("v3"), "v4": compiled.sha("v4")},
    )
    _dvo.OPS.append(op)
    _dvo._COMPILE_CACHE[(OP_NAME, "v3")] = compiled
    _dvo._COMPILE_CACHE[(OP_NAME, "v4")] = compiled
    return op


# ---------------------------------------------------------------- program
def _build_program(L):
    import concourse.bass as bass
    import concourse.mybir as mybir
    from concourse.ap import AP
    from concourse.library_overlay import lower_extended_insts

    bsa_op = _register_op()

    dt = mybir.dt.float32
    op = mybir.AluOpType

    nc = bass.Bass()
    sig_in = nc.declare_dram_parameter("sig_int", [128, T * NG], dt, isOutput=False)
    wt_in = nc.declare_dram_parameter("wt", [128, F + 1], dt, isOutput=False)
    out_d = nc.declare_dram_parameter("mout", [128, T * NG], dt, isOutput=True)

    ctxs = []

    def alloc(shape, dtype=dt):
        cm = nc.sbuf_tensor(shape, dtype)
        t = cm.__enter__()
        ctxs.append(cm)
        return t

    v = nc.vector

    W = T * NG                       # 16384
    SI = alloc([128, W])             # interleaved sig
    X = alloc([128, W])              # [20 pad | A0L -> masks] t-major
    Wt = alloc([128, F + 1])         # weights [-1, G[0..19]]
    tw = (2 * CH + HALO) * NG + 64
    tA = alloc([128, tw])
    tB = alloc([128, tw])

    sem_sig = [nc.alloc_semaphore(f"sig_dma{c}") for c in range(NCH)]
    sem_w = nc.alloc_semaphore("wt_load")
    sem_chain = nc.alloc_semaphore("chain")
    sem_out = nc.alloc_semaphore("out_dma")

    nc.sync.dma_start(out=Wt[:, :], in_=wt_in[:, :]).then_inc(sem_w, 16)
    for c in range(NCH):
        lo = c * CH * NG
        hi = min(W, (c + 1) * CH * NG)
        nc.sync.dma_start(out=SI[:, lo:hi], in_=sig_in[:, lo:hi]).then_inc(
            sem_sig[c], 16)

    def tree_range(lo_s, hi_s):
        """A0L = (20-window sums of sig) - L for steps [lo_s, min(hi_s,
        NSTEPS)), written into X at slot offset PAD*NG. All on DVE."""
        ns = hi_s - lo_s
        n_out = max(0, min(hi_s, NSTEPS) - lo_s)
        if n_out == 0:
            return
        halo = min(HALO, T - hi_s)
        w_in = (ns + halo) * NG
        last_step = lo_s + (w_in // NG) - 1
        for cc in range(lo_s // CH, min(last_step // CH, NCH - 1) + 1):
            v.wait_ge(sem_sig[cc], 16)
        base = lo_s * NG
        s = SI[:, base:base + w_in]
        w1 = w_in - 1 * NG
        v.tensor_tensor(out=tA[:, 0:w1], in0=s[:, 0:w1], in1=s[:, NG:w1 + NG], op=op.add)
        w2 = w1 - 2 * NG
        v.tensor_tensor(out=tB[:, 0:w2], in0=tA[:, 0:w2], in1=tA[:, 2 * NG:w2 + 2 * NG], op=op.add)
        w3 = w2 - 4 * NG
        v.tensor_tensor(out=tA[:, 0:w3], in0=tB[:, 0:w3], in1=tB[:, 4 * NG:w3 + 4 * NG], op=op.add)
        w4 = w3 - 8 * NG
        v.tensor_tensor(out=tA[:, 0:w4], in0=tA[:, 0:w4], in1=tA[:, 8 * NG:w4 + 8 * NG], op=op.add)
        wout = n_out * NG
        # A0 - L fused: (tA - L) + tB>>16
        v.scalar_tensor_tensor(
            out=X[:, PAD * NG + base:PAD * NG + base + wout],
            in0=tA[:, 0:wout], scalar=float(L),
            in1=tB[:, 16 * NG:16 * NG + wout],
            op0=op.subtract, op1=op.add,
        )

    v.memset(X[:, 0:PAD * NG], 0.0)
    v.wait_ge(sem_w, 16)
    tree_range(0, CH + 64)

    xh = X[:, :].tensor
    wh = Wt[:, :].tensor

    def chain_op(t0, k):
        npg = k * NG
        base = (PAD + t0) * NG
        in0 = AP(tensor=xh, offset=base, ap=[[W, 128], [1, npg], [-NG, F + 1]])
        in1 = AP(tensor=wh, offset=0, ap=[[F + 1, 128], [0, npg], [1, F + 1]])
        out = AP(tensor=xh, offset=base, ap=[[W, 128], [1, npg], [0, F + 1]])
        return v._custom_dve(bsa_op, out=out, in0=in0, in1=in1)

    nblk = (NSTEPS + CH - 1) // CH          # 8 chain chunks
    for cb in range(nblk):
        t_lo = cb * CH
        t_hi = min(NSTEPS, (cb + 1) * CH)
        t0 = t_lo
        last = None
        while t0 < t_hi:
            k = min(KSTEP, t_hi - t0)
            last = chain_op(t0, k)
            t0 += k
        last.then_inc(sem_chain, 1)
        if cb == 0:
            tree_range(CH + 64, 3 * CH + 64)
        elif cb in (2, 4):
            lo_t = (cb + 1) * CH + 64
            tree_range(lo_t, lo_t + 2 * CH)
        elif cb == 6:
            tree_range(7 * CH + 64, T)

    # masks out: out cols [0, NSTEPS*NG) <- X[PAD*NG:], zero tail <- X[0:PAD*NG]
    for cb in range(nblk):
        lo = cb * CH * NG
        hi = min(NSTEPS * NG, (cb + 1) * CH * NG)
        nc.sync.wait_ge(sem_chain, cb + 1)
        nc.sync.dma_start(out=out_d[:, lo:hi],
                          in_=X[:, PAD * NG + lo:PAD * NG + hi]).then_inc(sem_out, 16)
    nc.sync.dma_start(out=out_d[:, NSTEPS * NG:], in_=X[:, 0:PAD * NG]).then_inc(
        sem_out, 16)
    nc.sync.wait_ge(sem_out, 16 * (nblk + 1))
    lower_extended_insts(nc)
    return nc


def kernel(sig: np.ndarray, filt: np.ndarray) -> np.ndarray:
    from concourse.bass_utils import run_bass_kernel_spmd

    sig = np.ascontiguousarray(np.asarray(sig, dtype=np.float32))
    filt = np.asarray(filt, dtype=np.float32)
    assert sig.shape == (B, T) and filt.shape == (F,)

    fsum = np.float32(filt.sum())
    L = np.float32(fsum / np.float32(1.0 + THRESHOLD))
    G = np.cumsum(filt[::-1].astype(np.float64))[::-1].astype(np.float32)

    key = (filt.tobytes(),)
    if _CACHE.get("key") != key:
        _CACHE["nc"] = _build_program(L)
        _CACHE["key"] = key
    nc = _CACHE["nc"]

    wt = np.concatenate([[-1.0], G]).astype(np.float32)
    wt = np.broadcast_to(wt, (128, F + 1)).copy()

    in_maps = []
    for c in range(NCORES):
        blk = sig[c * RPC:(c + 1) * RPC]
        si = blk.reshape(NG, 128, T).transpose(1, 2, 0)
        si = np.ascontiguousarray(si.reshape(128, T * NG))
        in_maps.append({"sig_int": si, "wt": wt})

    res = run_bass_kernel_spmd(nc, in_maps, core_ids=list(range(NCORES)))

    out = np.empty((B, T), dtype=np.float32)
    for c in range(NCORES):
        m = res.results[c]["mout"].reshape(128, T, NG)
        out[c * RPC:(c + 1) * RPC] = np.ascontiguousarray(
            m.transpose(2, 0, 1).reshape(RPC, T))
    return out
